# revision 1
# baseline (speedup 1.0000x reference)
"""Block-diagonal linear layer (BlockLinearLayer) on 8 Trainium2 NeuronCores.

Math: x [65536, 4096] -> view [B, 128 blocks, 32]; out[b,n,j] = sum_k x3[b,n,k]*W[n,j,k] + bias
   -> out [65536, 1024].

Strategy (data-parallel over batch, 8 cores x 8192 rows):
- Host packs per-core x into a permuted layout xh[chunk, p, g*128+b] = x[128*chunk+b, 128*g+p]
  so that each 128-batch-row chunk is one fully-contiguous-per-partition [128, 4096] DMA
  with features on partitions (no on-device transpose needed).
- Host expands W [128, 8, 32] into a block-diagonal wd [128, 1024]: for feature group
  g (4 blocks = 128 features), wd[:, 32g:32g+32] is the [128, 32] block-diagonal matrix
  of those 4 blocks' weights.
- Device: per chunk, 32 matmuls (lhsT = x chunk slice [128 feat, 128 batch] stationary,
  rhs = wd slice [128, 32] moving) -> PSUM [128, 1024]; DVE adds bias while copying
  PSUM->SBUF; contiguous 512KB stores to the natural [8192, 1024] output layout.
"""

import os

import numpy as np

BATCH = 65536
INPUT_SIZE = 4096
OUTPUT_SIZE = 1024
N_BLOCKS = 128
BLOCK = 32
OPB = 8  # outputs per block
NCORES = 8
BC = BATCH // NCORES  # 8192 rows per core
P = 128
NCHUNK = BC // P  # 64 chunks of 128 batch rows
NGROUP = INPUT_SIZE // P  # 32 feature groups (4 blocks each)

LAST_EXEC_NS = None

_cached = None


def _build_program():
    import concourse.bass as bass
    import concourse.tile as tile
    from concourse import bacc, mybir
    from concourse.bass import ts

    f32 = mybir.dt.float32
    nc = bacc.Bacc("TRN2", target_bir_lowering=False, debug=False, num_devices=NCORES)

    xh = nc.dram_tensor("xh", [NCHUNK, P, INPUT_SIZE], f32, kind="ExternalInput").ap()
    wd = nc.dram_tensor("wd", [P, OUTPUT_SIZE], f32, kind="ExternalInput").ap()
    bias = nc.dram_tensor("bias", [P, OUTPUT_SIZE], f32, kind="ExternalInput").ap()
    out = nc.dram_tensor("out", [BC, OUTPUT_SIZE], f32, kind="ExternalOutput").ap()
    outv = out.rearrange("(c p) o -> c p o", p=P)

    with tile.TileContext(nc) as tc:
        with (
            tc.tile_pool(name="xpool", bufs=3) as xpool,
            tc.tile_pool(name="wpool", bufs=1) as wpool,
            tc.tile_pool(name="bpool", bufs=1) as bpool,
            tc.tile_pool(name="opool", bufs=3) as opool,
            tc.tile_pool(name="pspool", bufs=3, space="PSUM") as pspool,
        ):
            wtile = wpool.tile([P, OUTPUT_SIZE], f32)
            nc.sync.dma_start(wtile[:], wd)
            btile = bpool.tile([P, OUTPUT_SIZE], f32)
            nc.sync.dma_start(btile[:], bias)

            for c in range(NCHUNK):
                xt = xpool.tile([P, INPUT_SIZE], f32)
                nc.sync.dma_start(xt[:], xh[c])
                ps = pspool.tile([P, OUTPUT_SIZE], f32)
                for g in range(NGROUP):
                    nc.tensor.matmul(
                        ps[:, ts(g, BLOCK)],
                        xt[:, ts(g, P)],
                        wtile[:, ts(g, BLOCK)],
                        start=True,
                        stop=True,
                    )
                ot = opool.tile([P, OUTPUT_SIZE], f32)
                nc.vector.tensor_add(out=ot[:], in0=ps[:], in1=btile[:])
                nc.scalar.dma_start(outv[c], ot[:])

    nc.compile()
    return nc


def _host_pack_w(W: np.ndarray) -> np.ndarray:
    # wd[f, 32g + o]: for f = 32q + k, o = 8q + j -> W[4g + q, j, k]; else 0
    Wr = np.ascontiguousarray(W, dtype=np.float32).reshape(NGROUP, 4, OPB, BLOCK)
    Wd = np.zeros((NGROUP, P, BLOCK), dtype=np.float32)  # [g, f, o_local]
    for q in range(4):
        Wd[:, BLOCK * q : BLOCK * (q + 1), OPB * q : OPB * (q + 1)] = Wr[
            :, q
        ].transpose(0, 2, 1)
    return np.ascontiguousarray(Wd.transpose(1, 0, 2).reshape(P, OUTPUT_SIZE))


def _host_pack_x(xc: np.ndarray) -> np.ndarray:
    # xh[c, p, g*128 + b] = xc[128c + b, 128g + p]
    x4 = xc.reshape(NCHUNK, P, NGROUP, P)  # [c, b, g, p]
    return np.ascontiguousarray(x4.transpose(0, 3, 2, 1)).reshape(
        NCHUNK, P, INPUT_SIZE
    )


def kernel(x: np.ndarray, W: np.ndarray, b: np.ndarray) -> np.ndarray:
    global LAST_EXEC_NS, _cached
    from concourse.bass_utils import run_bass_kernel_spmd

    x = np.ascontiguousarray(x, dtype=np.float32)
    wd = _host_pack_w(W)
    brep = np.ascontiguousarray(
        np.broadcast_to(np.asarray(b, dtype=np.float32), (P, OUTPUT_SIZE))
    )

    if _cached is None:
        _cached = _build_program()
    nc = _cached

    in_maps = []
    for i in range(NCORES):
        xc = x[i * BC : (i + 1) * BC]
        in_maps.append({"xh": _host_pack_x(xc), "wd": wd, "bias": brep})

    trace = bool(os.environ.get("BLK_TRACE"))
    if trace:
        try:
            import ntff_shim  # noqa: F401
        except ImportError:
            trace = False
    res = run_bass_kernel_spmd(
        nc, in_maps, core_ids=list(range(NCORES)), trace=trace
    )
    LAST_EXEC_NS = res.exec_time_ns

    out = np.empty((BATCH, OUTPUT_SIZE), dtype=np.float32)
    for i in range(NCORES):
        out[i * BC : (i + 1) * BC] = res.results[i]["out"]
    return out


# revision 2
# speedup vs baseline: 1.7442x; 1.7442x over previous
"""Block-diagonal linear layer (BlockLinearLayer) on 8 Trainium2 NeuronCores.

Math: x [65536, 4096] -> view [B, 128 blocks, 32]; out[b,n,j] = sum_k x3[b,n,k]*W[n,j,k] + bias
   -> out [65536, 1024].

Strategy (data-parallel over batch, 8 cores x 8192 rows):
- W is expanded on host into block-diagonal [128, 32] tiles per feature group g
  (4 blocks = 128 features -> 32 outputs), stored as wd [128, 1024]. W is the
  *stationary* matmul operand (32-column LDWEIGHTS, cheap) and x streams as the
  moving operand at N=512 fp32 (PE ingests x at full stream rate).
- Host packs per-core x so each DMA is fully contiguous per partition with the
  feature group on partitions: xq[q, s, p, gg*1024 + b] = x[1024*s + b, 512*q + 128*gg + p].
- Output lands transposed in PSUM ([32 outs, 512 batch] per matmul); four groups
  (one "quad" q) stack into the 128 PSUM partitions via col-tiling
  (tile_position=(0, 32*gg)). DVE adds per-partition bias while copying
  PSUM->SBUF; 2 MiB contiguous stores write outT [1024, 8192] per core; host
  transposes outT back (cheap: output is 4x smaller than input).
"""

import os

import numpy as np

BATCH = 65536
INPUT_SIZE = 4096
OUTPUT_SIZE = 1024
N_BLOCKS = 128
BLOCK = 32
OPB = 8  # outputs per block
NCORES = 8
BC = BATCH // NCORES  # 8192 rows per core
P = 128
NQ = 8  # quads (4 feature groups each -> 128 output rows)
NS = 8  # batch strips of 1024 per core
SB = 1024  # strip batch size

LAST_EXEC_NS = None

_cached = None


def _build_program():
    import concourse.bass as bass
    import concourse.tile as tile
    from concourse import bacc, mybir
    from concourse.bass import ts

    f32 = mybir.dt.float32
    nc = bacc.Bacc("TRN2", target_bir_lowering=False, debug=False, num_devices=NCORES)

    xq = nc.dram_tensor("xq", [NQ, NS, P, 4096], f32, kind="ExternalInput").ap()
    wd = nc.dram_tensor("wd", [P, OUTPUT_SIZE], f32, kind="ExternalInput").ap()
    biasT = nc.dram_tensor("biasT", [P, NQ], f32, kind="ExternalInput").ap()
    outT = nc.dram_tensor("outT", [OUTPUT_SIZE, BC], f32, kind="ExternalOutput").ap()
    # outT[128q + p, 4096*half + m] -> [q, half, p, m]
    outTv = outT.rearrange("(q p) (half m) -> q half p m", p=P, m=4096)

    with tile.TileContext(nc) as tc:
        with (
            tc.tile_pool(name="xpool", bufs=3) as xpool,
            tc.tile_pool(name="wpool", bufs=1) as wpool,
            tc.tile_pool(name="bpool", bufs=1) as bpool,
            tc.tile_pool(name="opool", bufs=2) as opool,
            tc.tile_pool(name="pspool", bufs=3, space="PSUM") as pspool,
        ):
            wtile = wpool.tile([P, OUTPUT_SIZE], f32)
            nc.sync.dma_start(wtile[:], wd)
            btile = bpool.tile([P, NQ], f32)
            nc.sync.dma_start(btile[:], biasT)

            for q in range(NQ):
                ot = None
                for s in range(NS):
                    xt = xpool.tile([P, 4096], f32)
                    nc.sync.dma_start(xt[:], xq[q, s])
                    ps = pspool.tile([P, SB], f32)
                    for gg in range(4):
                        g = 4 * q + gg
                        for h in range(2):
                            nc.tensor.matmul(
                                ps[32 * gg : 32 * (gg + 1), ts(h, 512)],
                                wtile[:, ts(g, BLOCK)],
                                xt[:, 1024 * gg + 512 * h : 1024 * gg + 512 * (h + 1)],
                                start=True,
                                stop=True,
                                tile_position=(0, 32 * gg),
                            )
                    if s % 4 == 0:
                        ot = opool.tile([P, 4096], f32)
                    nc.vector.tensor_scalar_add(
                        out=ot[:, ts(s % 4, SB)],
                        in0=ps[:],
                        scalar1=btile[:, q : q + 1],
                    )
                    if s % 4 == 3:
                        nc.scalar.dma_start(outTv[q, s // 4], ot[:])

    nc.compile()
    return nc


def _host_pack_w(W: np.ndarray) -> np.ndarray:
    # wd[f, 32g + o]: for f = 32qq + k, o = 8qq + j -> W[4g + qq, j, k]; else 0
    NGROUP = 32
    Wr = np.ascontiguousarray(W, dtype=np.float32).reshape(NGROUP, 4, OPB, BLOCK)
    Wd = np.zeros((NGROUP, P, BLOCK), dtype=np.float32)  # [g, f, o_local]
    for qq in range(4):
        Wd[:, BLOCK * qq : BLOCK * (qq + 1), OPB * qq : OPB * (qq + 1)] = Wr[
            :, qq
        ].transpose(0, 2, 1)
    return np.ascontiguousarray(Wd.transpose(1, 0, 2).reshape(P, OUTPUT_SIZE))


def _host_pack_x(xc: np.ndarray) -> np.ndarray:
    # xq[q, s, p, gg*1024 + b] = xc[1024*s + b, 512*q + 128*gg + p]
    x5 = xc.reshape(NS, SB, NQ, 4, P)  # [s, b, q, gg, p]
    return np.ascontiguousarray(x5.transpose(2, 0, 4, 3, 1)).reshape(NQ, NS, P, 4096)


def kernel(x: np.ndarray, W: np.ndarray, b: np.ndarray) -> np.ndarray:
    global LAST_EXEC_NS, _cached
    from concourse.bass_utils import run_bass_kernel_spmd

    x = np.ascontiguousarray(x, dtype=np.float32)
    wd = _host_pack_w(W)
    bT = np.ascontiguousarray(
        np.asarray(b, dtype=np.float32).reshape(NQ, P).T
    )  # [128, 8]

    if _cached is None:
        _cached = _build_program()
    nc = _cached

    in_maps = []
    for i in range(NCORES):
        xc = x[i * BC : (i + 1) * BC]
        in_maps.append({"xq": _host_pack_x(xc), "wd": wd, "biasT": bT})

    trace = bool(os.environ.get("BLK_TRACE"))
    if trace:
        try:
            import ntff_shim  # noqa: F401
        except ImportError:
            trace = False
    res = run_bass_kernel_spmd(nc, in_maps, core_ids=list(range(NCORES)), trace=trace)
    LAST_EXEC_NS = res.exec_time_ns

    out = np.empty((BATCH, OUTPUT_SIZE), dtype=np.float32)
    for i in range(NCORES):
        out[i * BC : (i + 1) * BC] = res.results[i]["outT"].T
    return out


# revision 3
# speedup vs baseline: 1.7867x; 1.0244x over previous
"""Block-diagonal linear layer (BlockLinearLayer) on 8 Trainium2 NeuronCores.

Math: x [65536, 4096] -> view [B, 128 blocks, 32]; out[b,n,j] = sum_k x3[b,n,k]*W[n,j,k] + bias
   -> out [65536, 1024].

Strategy (data-parallel over batch, 8 cores x 8192 rows):
- W is expanded on host into block-diagonal [128, 32] tiles per feature group g
  (4 blocks = 128 features -> 32 outputs), stored as wd [128, 1024]. W is the
  *stationary* matmul operand (32-column LDWEIGHTS, cheap) and x streams as the
  moving operand at N=512 fp32 (PE ingests x at full stream rate).
- Host packs per-core x so each 4 MiB DMA is fully contiguous per partition
  (32 KiB runs) with the feature group on partitions:
  xq[q, s, p, gg*2048 + b] = x[2048*s + b, 512*q + 128*gg + p].
- Output lands transposed in PSUM ([32 outs, 512 batch] per matmul); four groups
  (one "quad" q) stack into the 128 PSUM partitions via col-tiling
  (tile_position=(0, 32*gg)). DVE adds per-partition bias while copying
  PSUM->SBUF; one 4 MiB contiguous store per quad writes outT [1024, 8192];
  host transposes outT back (cheap: output is 4x smaller than input).
"""

import os

import numpy as np

BATCH = 65536
INPUT_SIZE = 4096
OUTPUT_SIZE = 1024
N_BLOCKS = 128
BLOCK = 32
OPB = 8  # outputs per block
NCORES = 8
BC = BATCH // NCORES  # 8192 rows per core
P = 128
NQ = 8  # quads (4 feature groups each -> 128 output rows)
NS = 4  # batch strips per core
SB = 2048  # strip batch size

LAST_EXEC_NS = None

_cached = None


def _build_program():
    import concourse.bass as bass
    import concourse.tile as tile
    from concourse import bacc, mybir
    from concourse.bass import ts

    f32 = mybir.dt.float32
    nc = bacc.Bacc("TRN2", target_bir_lowering=False, debug=False, num_devices=NCORES)

    xq = nc.dram_tensor("xq", [NQ, NS, P, 4 * SB], f32, kind="ExternalInput").ap()
    wd = nc.dram_tensor("wd", [P, OUTPUT_SIZE], f32, kind="ExternalInput").ap()
    biasT = nc.dram_tensor("biasT", [P, NQ], f32, kind="ExternalInput").ap()
    outT = nc.dram_tensor("outT", [OUTPUT_SIZE, BC], f32, kind="ExternalOutput").ap()
    outTv = outT.rearrange("(q p) m -> q p m", p=P)  # [8, 128, 8192]

    with tile.TileContext(nc) as tc:
        with (
            tc.tile_pool(name="xpool", bufs=3) as xpool,
            tc.tile_pool(name="wpool", bufs=1) as wpool,
            tc.tile_pool(name="bpool", bufs=1) as bpool,
            tc.tile_pool(name="opool", bufs=2) as opool,
            tc.tile_pool(name="pspool", bufs=3, space="PSUM") as pspool,
        ):
            wtile = wpool.tile([P, OUTPUT_SIZE], f32)
            nc.sync.dma_start(wtile[:], wd)
            btile = bpool.tile([P, NQ], f32)
            nc.sync.dma_start(btile[:], biasT)

            for q in range(NQ):
                ot = opool.tile([P, BC], f32)
                for s in range(NS):
                    xt = xpool.tile([P, 4 * SB], f32)
                    nc.sync.dma_start(xt[:], xq[q, s])
                    for hh in range(SB // 1024):
                        ps = pspool.tile([P, 1024], f32)
                        for gg in range(4):
                            for hb in range(2):
                                h = 2 * hh + hb
                                nc.tensor.matmul(
                                    ps[32 * gg : 32 * (gg + 1), ts(hb, 512)],
                                    wtile[:, ts(4 * q + gg, BLOCK)],
                                    xt[:, 2048 * gg + 512 * h : 2048 * gg + 512 * (h + 1)],
                                    start=True,
                                    stop=True,
                                    tile_position=(0, 32 * gg),
                                )
                        nc.vector.tensor_scalar_add(
                            out=ot[:, SB * s + 1024 * hh : SB * s + 1024 * (hh + 1)],
                            in0=ps[:],
                            scalar1=btile[:, q : q + 1],
                        )
                nc.scalar.dma_start(outTv[q], ot[:])

    nc.compile()
    return nc


def _host_pack_w(W: np.ndarray) -> np.ndarray:
    # wd[f, 32g + o]: for f = 32qq + k, o = 8qq + j -> W[4g + qq, j, k]; else 0
    NGROUP = 32
    Wr = np.ascontiguousarray(W, dtype=np.float32).reshape(NGROUP, 4, OPB, BLOCK)
    Wd = np.zeros((NGROUP, P, BLOCK), dtype=np.float32)  # [g, f, o_local]
    for qq in range(4):
        Wd[:, BLOCK * qq : BLOCK * (qq + 1), OPB * qq : OPB * (qq + 1)] = Wr[
            :, qq
        ].transpose(0, 2, 1)
    return np.ascontiguousarray(Wd.transpose(1, 0, 2).reshape(P, OUTPUT_SIZE))


def _host_pack_x(xc: np.ndarray) -> np.ndarray:
    # xq[q, s, p, gg*SB + b] = xc[SB*s + b, 512*q + 128*gg + p]
    x5 = xc.reshape(NS, SB, NQ, 4, P)  # [s, b, q, gg, p]
    return np.ascontiguousarray(x5.transpose(2, 0, 4, 3, 1)).reshape(NQ, NS, P, 4 * SB)


def kernel(x: np.ndarray, W: np.ndarray, b: np.ndarray) -> np.ndarray:
    global LAST_EXEC_NS, _cached
    from concourse.bass_utils import run_bass_kernel_spmd

    x = np.ascontiguousarray(x, dtype=np.float32)
    wd = _host_pack_w(W)
    bT = np.ascontiguousarray(
        np.asarray(b, dtype=np.float32).reshape(NQ, P).T
    )  # [128, 8]

    if _cached is None:
        _cached = _build_program()
    nc = _cached

    in_maps = []
    for i in range(NCORES):
        xc = x[i * BC : (i + 1) * BC]
        in_maps.append({"xq": _host_pack_x(xc), "wd": wd, "biasT": bT})

    trace = bool(os.environ.get("BLK_TRACE"))
    if trace:
        try:
            import ntff_shim  # noqa: F401
        except ImportError:
            trace = False
    res = run_bass_kernel_spmd(nc, in_maps, core_ids=list(range(NCORES)), trace=trace)
    LAST_EXEC_NS = res.exec_time_ns

    out = np.empty((BATCH, OUTPUT_SIZE), dtype=np.float32)
    for i in range(NCORES):
        out[i * BC : (i + 1) * BC] = res.results[i]["outT"].T
    return out


# revision 4
# speedup vs baseline: 1.8526x; 1.0369x over previous
"""Block-diagonal linear layer (BlockLinearLayer) on 8 Trainium2 NeuronCores.

Math: x [65536, 4096] -> view [B, 128 blocks, 32]; out[b,n,j] = sum_k x3[b,n,k]*W[n,j,k] + bias
   -> out [65536, 1024].

Strategy (data-parallel over batch, 8 cores x 8192 rows):
- W is expanded on host into block-diagonal [128, 32] tiles per feature group g
  (4 blocks = 128 features -> 32 outputs), stored as wd [128, 1024]. W is the
  *stationary* matmul operand (32-column LDWEIGHTS, cheap) and x streams as the
  moving operand at N=512 fp32 (PE ingests x at full stream rate).
- Host packs per-core x so each 2 MiB DMA is fully contiguous per partition
  (16 KiB runs) with the feature group on partitions:
  xq[q, s, p, gg*1024 + b] = x[1024*s + b, 512*q + 128*gg + p].
- Output lands transposed in PSUM ([32 outs, 512 batch] per matmul); four groups
  (one "quad" q) stack into the 128 PSUM partitions via col-tiling
  (tile_position=(0, 32*gg)). DVE adds per-partition bias while copying
  PSUM->SBUF; 1 MiB contiguous stores write outT [1024, 8192] per core; host
  transposes outT back (cheap: output is 4x smaller than input).
- Deep DMA lookahead (8 x-tile buffers) keeps the HWDGE descriptor queues fed,
  since the SDMA engines run ~27 GB/s per packet and stall on descriptor gaps.
"""

import os

import numpy as np

BATCH = 65536
INPUT_SIZE = 4096
OUTPUT_SIZE = 1024
N_BLOCKS = 128
BLOCK = 32
OPB = 8  # outputs per block
NCORES = 8
BC = BATCH // NCORES  # 8192 rows per core
P = 128
NQ = 8  # quads (4 feature groups each -> 128 output rows)
NS = 8  # batch strips per core
SB = 1024  # strip batch size

LAST_EXEC_NS = None

_cached = None


def _build_program():
    import concourse.bass as bass
    import concourse.tile as tile
    from concourse import bacc, mybir
    from concourse.bass import ts

    f32 = mybir.dt.float32
    nc = bacc.Bacc("TRN2", target_bir_lowering=False, debug=False, num_devices=NCORES)

    xq = nc.dram_tensor("xq", [NQ, NS, P, 4 * SB], f32, kind="ExternalInput").ap()
    wd = nc.dram_tensor("wd", [P, OUTPUT_SIZE], f32, kind="ExternalInput").ap()
    biasT = nc.dram_tensor("biasT", [P, NQ], f32, kind="ExternalInput").ap()
    outT = nc.dram_tensor("outT", [OUTPUT_SIZE, BC], f32, kind="ExternalOutput").ap()
    outTv = outT.rearrange("(q p) m -> q p m", p=P)  # [8, 128, 8192]

    with tile.TileContext(nc) as tc:
        with (
            tc.tile_pool(name="xpool", bufs=8) as xpool,
            tc.tile_pool(name="wpool", bufs=1) as wpool,
            tc.tile_pool(name="bpool", bufs=1) as bpool,
            tc.tile_pool(name="opool", bufs=4) as opool,
            tc.tile_pool(name="pspool", bufs=3, space="PSUM") as pspool,
        ):
            wtile = wpool.tile([P, OUTPUT_SIZE], f32)
            nc.sync.dma_start(wtile[:], wd)
            btile = bpool.tile([P, NQ], f32)
            nc.sync.dma_start(btile[:], biasT)

            for q in range(NQ):
                ot = None
                for s in range(NS):
                    xt = xpool.tile([P, 4 * SB], f32)
                    nc.sync.dma_start(xt[:], xq[q, s])
                    ps = pspool.tile([P, SB], f32)
                    for gg in range(4):
                        for h in range(2):
                            nc.tensor.matmul(
                                ps[32 * gg : 32 * (gg + 1), ts(h, 512)],
                                wtile[:, ts(4 * q + gg, BLOCK)],
                                xt[:, SB * gg + 512 * h : SB * gg + 512 * (h + 1)],
                                start=True,
                                stop=True,
                                tile_position=(0, 32 * gg),
                            )
                    if s % 2 == 0:
                        ot = opool.tile([P, 2 * SB], f32)
                    nc.vector.tensor_scalar_add(
                        out=ot[:, ts(s % 2, SB)],
                        in0=ps[:],
                        scalar1=btile[:, q : q + 1],
                    )
                    if s % 2 == 1:
                        nc.scalar.dma_start(
                            outTv[q][:, 2 * SB * (s // 2) : 2 * SB * (s // 2 + 1)],
                            ot[:],
                        )

    nc.compile()
    return nc


def _host_pack_w(W: np.ndarray) -> np.ndarray:
    # wd[f, 32g + o]: for f = 32qq + k, o = 8qq + j -> W[4g + qq, j, k]; else 0
    NGROUP = 32
    Wr = np.ascontiguousarray(W, dtype=np.float32).reshape(NGROUP, 4, OPB, BLOCK)
    Wd = np.zeros((NGROUP, P, BLOCK), dtype=np.float32)  # [g, f, o_local]
    for qq in range(4):
        Wd[:, BLOCK * qq : BLOCK * (qq + 1), OPB * qq : OPB * (qq + 1)] = Wr[
            :, qq
        ].transpose(0, 2, 1)
    return np.ascontiguousarray(Wd.transpose(1, 0, 2).reshape(P, OUTPUT_SIZE))


def _host_pack_x(xc: np.ndarray) -> np.ndarray:
    # xq[q, s, p, gg*SB + b] = xc[SB*s + b, 512*q + 128*gg + p]
    x5 = xc.reshape(NS, SB, NQ, 4, P)  # [s, b, q, gg, p]
    return np.ascontiguousarray(x5.transpose(2, 0, 4, 3, 1)).reshape(NQ, NS, P, 4 * SB)


def kernel(x: np.ndarray, W: np.ndarray, b: np.ndarray) -> np.ndarray:
    global LAST_EXEC_NS, _cached
    from concourse.bass_utils import run_bass_kernel_spmd

    x = np.ascontiguousarray(x, dtype=np.float32)
    wd = _host_pack_w(W)
    bT = np.ascontiguousarray(
        np.asarray(b, dtype=np.float32).reshape(NQ, P).T
    )  # [128, 8]

    if _cached is None:
        _cached = _build_program()
    nc = _cached

    in_maps = []
    for i in range(NCORES):
        xc = x[i * BC : (i + 1) * BC]
        in_maps.append({"xq": _host_pack_x(xc), "wd": wd, "biasT": bT})

    trace = bool(os.environ.get("BLK_TRACE"))
    if trace:
        try:
            import ntff_shim  # noqa: F401
        except ImportError:
            trace = False
    res = run_bass_kernel_spmd(nc, in_maps, core_ids=list(range(NCORES)), trace=trace)
    LAST_EXEC_NS = res.exec_time_ns

    out = np.empty((BATCH, OUTPUT_SIZE), dtype=np.float32)
    for i in range(NCORES):
        out[i * BC : (i + 1) * BC] = res.results[i]["outT"].T
    return out
